# revision 11
# baseline (speedup 1.0000x reference)
"""Multi-Head Latent Attention (MLA) Trainium2 kernel.

Problem: B=4, T=2048, C=768, H=12, D=64, R=64, causal attention, RoPE.
Sharding: 8 cores = 4 batches x 2 head-groups (6 heads each). Each core
computes a partial output y_partial[b] = (attn_out_heads @ Wo_slice); host
sums the two head-group partials per batch and adds the bias.

All on-chip compute in bf16 (matmuls) with fp32 PSUM accumulation; softmax
denominators handled flash-style: exp(S) unnormalized, denominator obtained
via a ones-column appended to V, division folded into the attention-output
eviction.

RoPE: rot(q) is produced by a second projection with host-permuted/negated
weight columns (rot(W^T x) = (W P^T)^T x), so no on-chip partition shifts.
"""
import numpy as np
import ml_dtypes

import concourse.bass as bass
import concourse.mybir as mybir
import concourse.tile as tile
from concourse.bass_utils import run_bass_kernel_spmd
from concourse.masks import make_identity

BF16 = mybir.dt.bfloat16
F32 = mybir.dt.float32

B, T, C = 4, 2048, 768
H, D = 12, 64
R = 64
HL = 6              # heads per core
ROPE_THETA = 10000.0
N_CORES = 8

_cached = {}

_CTRL_TYPES = ("InstDrain", "InstNoOp", "InstEventSemaphore", "InstAllEngineBarrier")


def _split_sync_waits(nc, max_waits=1):
    """Split instructions carrying >1 sem wait into wait-carrier NoOps
    (this walrus build supports a single sync wait per instruction)."""
    for f in nc.m.functions:
        for bb in f.blocks:
            new_list, changed = [], False
            for ins in bb.instructions:
                si = ins.sync_info
                waits = list(si.on_wait) if si is not None else []
                if len(waits) > max_waits:
                    excess, keep = waits[:-max_waits], waits[-max_waits:]
                    for i in range(0, len(excess), max_waits):
                        nop = mybir.InstNoOp(
                            name=f"waitsplit-{nc.next_id()}",
                            engine=ins.engine, ins=[], outs=[],
                            sync_info=mybir.SyncInfo(
                                on_wait=excess[i:i + max_waits], on_update=[]))
                        nc.register_instruction(nop)
                        new_list.append(nop)
                    ins.sync_info = mybir.SyncInfo(
                        on_wait=keep, on_update=list(si.on_update))
                    changed = True
                new_list.append(ins)
            if changed:
                bb.instructions = new_list


def _build_nc():
    nc = bass.Bass("TRN2", target_bir_lowering=False)

    # ---- DRAM I/O ----
    xT_d = nc.dram_tensor("xT", [C, T], BF16, kind="ExternalInput")
    wq_d = nc.dram_tensor("wq", [C, HL * D], BF16, kind="ExternalInput")
    wqr_d = nc.dram_tensor("wqr", [C, HL * D], BF16, kind="ExternalInput")
    wdown_d = nc.dram_tensor("wdown", [C, R], BF16, kind="ExternalInput")
    wupk_d = nc.dram_tensor("wupk", [R, HL * D], BF16, kind="ExternalInput")
    wupkr_d = nc.dram_tensor("wupkr", [R, HL * D], BF16, kind="ExternalInput")
    wupv_d = nc.dram_tensor("wupv", [R, HL * D], BF16, kind="ExternalInput")
    wo_d = nc.dram_tensor("wo", [HL * D, C], BF16, kind="ExternalInput")
    cos_d = nc.dram_tensor("cosT", [D, T], BF16, kind="ExternalInput")
    sin_d = nc.dram_tensor("sinT", [D, T], BF16, kind="ExternalInput")
    lng_d = nc.dram_tensor("lng", [R, 1], F32, kind="ExternalInput")
    lnb_d = nc.dram_tensor("lnb", [R, 1], F32, kind="ExternalInput")
    mask_d = nc.dram_tensor("mask128", [128, 128], BF16, kind="ExternalInput")
    y_d = nc.dram_tensor("y", [T, C], F32, kind="ExternalOutput")

    TT = T // 128    # 16 token tiles
    CC = C // 128    # 6 contraction chunks
    QC = T // 512    # 4 q chunks of 512

    with tile.TileContext(nc) as tc:
        with tc.tile_pool(name="persist", bufs=1) as pp:
            # weights / constants
            wq_t = pp.tile([128, CC, HL * D], BF16)
            wqr_t = pp.tile([128, CC, HL * D], BF16)
            wdown_t = pp.tile([128, CC, R], BF16)
            wupk_t = pp.tile([R, HL * D], BF16)
            wupkr_t = pp.tile([R, HL * D], BF16)
            wupv_t = pp.tile([R, HL * D], BF16)
            wo_t = pp.tile([R, HL, C], BF16)   # per-head [64, 768]
            cos_t = pp.tile([D, T], BF16)
            sin_t = pp.tile([D, T], BF16)
            lng_t = pp.tile([R, 1], F32)
            lnb_t = pp.tile([R, 1], F32)
            mask_t = pp.tile([128, 128], BF16)
            ident_t = pp.tile([128, 128], BF16)
            xT_t = pp.tile([128, CC, T], BF16)
            ckvT_t = pp.tile([R, T], BF16)

            for cc in range(CC):
                nc.sync.dma_start(wq_t[:, cc, :], wq_d[cc * 128:(cc + 1) * 128, :])
                nc.sync.dma_start(wqr_t[:, cc, :], wqr_d[cc * 128:(cc + 1) * 128, :])
                nc.sync.dma_start(wdown_t[:, cc, :], wdown_d[cc * 128:(cc + 1) * 128, :])
                nc.sync.dma_start(xT_t[:, cc, :], xT_d[cc * 128:(cc + 1) * 128, :])
            nc.sync.dma_start(wupk_t[:, :], wupk_d[:, :])
            nc.sync.dma_start(wupkr_t[:, :], wupkr_d[:, :])
            nc.sync.dma_start(wupv_t[:, :], wupv_d[:, :])
            for h in range(HL):
                nc.sync.dma_start(wo_t[:, h, :], wo_d[h * D:(h + 1) * D, :])
            nc.sync.dma_start(cos_t[:, :], cos_d[:, :])
            nc.sync.dma_start(sin_t[:, :], sin_d[:, :])
            nc.sync.dma_start(lng_t[:, :], lng_d[:, :])
            nc.sync.dma_start(lnb_t[:, :], lnb_d[:, :])
            nc.sync.dma_start(mask_t[:, :], mask_d[:, :])
            make_identity(nc, ident_t[:, :])

            # per-head persistent activations
            dn_t = pp.tile([2 * HL, 1024], F32)
            rcb_t = pp.tile([2 * HL, 1024], BF16)
            rrow = [pp.tile([1, 1024], BF16, name=f"rrow{j}") for j in range(2 * HL)]
            ones1 = pp.tile([1, D], BF16)
            qT_h = [pp.tile([D, T], BF16, name=f"qT{h}") for h in range(HL)]
            kT_h = [pp.tile([D, T], BF16, name=f"kT{h}") for h in range(HL)]
            v_h = [pp.tile([128, TT, D + 1], BF16, name=f"v{h}") for h in range(HL)]
            oT_h = [pp.tile([D, T], BF16, name=f"oT{h}") for h in range(HL)]

            for h in range(HL):
                nc.gpsimd.memset(v_h[h][:, :, D:D + 1], 1.0)
            nc.gpsimd.memset(ones1[:, :], 1.0)

            # ---------------- Phase A: projections ----------------
            with tc.tile_pool(name="psA", bufs=2, space="PSUM") as psA, \
                 tc.tile_pool(name="sbA", bufs=3) as sbA:

                # ckv = LN(x @ Wdown), then transpose -> ckvT [R, T]
                for it in range(TT):
                    ps_c = psA.tile([128, R], F32, tag="ckv")
                    for cc in range(CC):
                        nc.tensor.matmul(
                            ps_c[:, :],
                            xT_t[:, cc, it * 128:(it + 1) * 128],
                            wdown_t[:, cc, :],
                            start=(cc == 0), stop=(cc == CC - 1))
                    mu = sbA.tile([128, 1], F32, tag="mu")
                    nc.vector.reduce_sum(mu[:, :], ps_c[:, :], axis=mybir.AxisListType.X)
                    nc.vector.tensor_scalar_mul(mu[:, :], mu[:, :], 1.0 / R)
                    cen = sbA.tile([128, R], F32, tag="cen")
                    nc.vector.tensor_scalar(
                        cen[:, :], ps_c[:, :], mu[:, :], None,
                        op0=mybir.AluOpType.subtract)
                    sq = sbA.tile([128, R], F32, tag="sq")
                    vs = sbA.tile([128, 1], F32, tag="vs")
                    nc.vector.scalar_tensor_tensor(
                        sq[:, :], cen[:, :], 1.0, cen[:, :],
                        op0=mybir.AluOpType.bypass, op1=mybir.AluOpType.mult,
                        accum_out=vs[:, :])
                    nc.vector.tensor_scalar(
                        vs[:, :], vs[:, :], 1.0 / R, 1e-5,
                        op0=mybir.AluOpType.mult, op1=mybir.AluOpType.add)
                    # rstd = exp(-0.5 * ln(var + 1e-5))  (stays in one ACT table set)
                    nc.scalar.activation(vs[:, :], vs[:, :],
                                         mybir.ActivationFunctionType.Ln)
                    nc.scalar.activation(vs[:, :], vs[:, :],
                                         mybir.ActivationFunctionType.Exp, scale=-0.5)
                    ckvn = sbA.tile([128, R], BF16, tag="ckvn")
                    nc.vector.tensor_scalar(
                        ckvn[:, :], cen[:, :], vs[:, :], None,
                        op0=mybir.AluOpType.mult)
                    ps_ct = psA.tile([R, 128], BF16, tag="ckvT")
                    nc.tensor.transpose(ps_ct[:, :], ckvn[:, :], ident_t[:, :])
                    nc.vector.tensor_scalar(
                        ckvT_t[:, it * 128:(it + 1) * 128], ps_ct[:, :],
                        lng_t[:, :], lnb_t[:, :],
                        op0=mybir.AluOpType.mult, op1=mybir.AluOpType.add)

                # q projection + rope (per head, 512-wide chunks)
                for h in range(HL):
                    hs = slice(h * D, (h + 1) * D)
                    for jc in range(QC):
                        qs = slice(jc * 512, (jc + 1) * 512)
                        ps_q = psA.tile([D, 512], F32, tag="q")
                        ps_qr = psA.tile([D, 512], F32, tag="qr")
                        for cc in range(CC):
                            nc.tensor.matmul(
                                ps_q[:, :], wq_t[:, cc, hs], xT_t[:, cc, qs],
                                start=(cc == 0), stop=(cc == CC - 1))
                        for cc in range(CC):
                            nc.tensor.matmul(
                                ps_qr[:, :], wqr_t[:, cc, hs], xT_t[:, cc, qs],
                                start=(cc == 0), stop=(cc == CC - 1))
                        t1 = sbA.tile([D, 512], BF16, tag="t1")
                        t2 = sbA.tile([D, 512], BF16, tag="t2")
                        nc.vector.tensor_mul(t1[:, :], ps_q[:, :], cos_t[:, qs])
                        nc.vector.tensor_mul(t2[:, :], ps_qr[:, :], sin_t[:, qs])
                        nc.vector.tensor_add(qT_h[h][:, qs], t1[:, :], t2[:, :])

                # k projection + rope
                for h in range(HL):
                    hs = slice(h * D, (h + 1) * D)
                    for jc in range(QC):
                        qs = slice(jc * 512, (jc + 1) * 512)
                        ps_k = psA.tile([D, 512], F32, tag="q")
                        ps_kr = psA.tile([D, 512], F32, tag="qr")
                        nc.tensor.matmul(ps_k[:, :], wupk_t[:, hs], ckvT_t[:, qs],
                                         start=True, stop=True)
                        nc.tensor.matmul(ps_kr[:, :], wupkr_t[:, hs], ckvT_t[:, qs],
                                         start=True, stop=True)
                        t1 = sbA.tile([D, 512], BF16, tag="t1")
                        t2 = sbA.tile([D, 512], BF16, tag="t2")
                        nc.vector.tensor_mul(t1[:, :], ps_k[:, :], cos_t[:, qs])
                        nc.vector.tensor_mul(t2[:, :], ps_kr[:, :], sin_t[:, qs])
                        nc.vector.tensor_add(kT_h[h][:, qs], t1[:, :], t2[:, :])

                # v projection (natural layout [t, d] = AV lhsT)
                for h in range(HL):
                    hs = slice(h * D, (h + 1) * D)
                    for it in range(TT):
                        ps_v = psA.tile([128, D], F32, tag="ckv")
                        nc.tensor.matmul(
                            ps_v[:, :], ckvT_t[:, it * 128:(it + 1) * 128],
                            wupv_t[:, hs], start=True, stop=True)
                        nc.vector.tensor_copy(v_h[h][:, it, 0:D], ps_v[:, :])

            # ---------------- Phase B: attention ----------------
            with tc.tile_pool(name="psB", bufs=2, space="PSUM") as psB, \
                 tc.tile_pool(name="sbB", bufs=3) as sbB:
                for h in range(HL):
                    for jq in range(2):          # 1024-wide q chunks
                        q0 = jq * 1024
                        ps_o = psB.tile([D + 1, 1024], F32, tag="o")
                        last_i = 8 * jq + 7
                        for i in range(last_i + 1):
                            qlo = max(128 * i, q0)
                            width = q0 + 1024 - qlo
                            ps_s = psB.tile([128, 1024], F32, tag="s")
                            # S^T matmuls in <=512 moving chunks
                            off = 0
                            while off < width:
                                w = min(512, width - off)
                                nc.tensor.matmul(
                                    ps_s[:, off:off + w],
                                    kT_h[h][:, 128 * i:128 * (i + 1)],
                                    qT_h[h][:, qlo + off:qlo + off + w],
                                    start=True, stop=True)
                                off += w
                            pT = sbB.tile([128, 1024], BF16, tag="pT")
                            nc.scalar.activation(
                                pT[:, 0:width], ps_s[:, 0:width],
                                mybir.ActivationFunctionType.Exp, scale=float(D) ** -0.5)
                            if 128 * i >= q0:
                                # diagonal tile: mask first 128 cols (q-k in [0,128))
                                nc.vector.tensor_mul(
                                    pT[:, 0:128], pT[:, 0:128], mask_t[:, :])
                            off = 0
                            while off < width:
                                pos = qlo - q0 + off
                                w = min(512 - (pos % 512), width - off)
                                nc.tensor.matmul(
                                    ps_o[:, pos:pos + w],
                                    v_h[h][:, i, :],
                                    pT[:, off:off + w],
                                    start=(i == 0), stop=(i == last_i),
                                    skip_group_check=True)
                                off += w
                        # stash unnormalized output + denominator row
                        dnrow = sbB.tile([1, 1024], F32, tag="dn")
                        nc.vector.tensor_copy(dnrow[:, :], ps_o[D:D + 1, :])
                        nc.sync.dma_start(
                            dn_t[2 * h + jq:2 * h + jq + 1, :], dnrow[:, :])
                        nc.vector.tensor_copy(
                            oT_h[h][:, q0:q0 + 1024], ps_o[0:D, :])

            # ---------------- Phase B2: softmax normalization ----------------
            with tc.tile_pool(name="psB2", bufs=2, space="PSUM") as psB2:
                rc_t = pp.tile([2 * HL, 1024], F32)
                nc.vector.reciprocal(rc_t[:, :], dn_t[:, :])
                nc.vector.tensor_copy(rcb_t[:, :], rc_t[:, :])
                for j in range(2 * HL):
                    nc.sync.dma_start(rrow[j][:, :], rcb_t[j:j + 1, :])
                for h in range(HL):
                    for jq in range(2):
                        q0 = jq * 1024
                        rb_ps = psB2.tile([D, 1024], F32, tag="rb")
                        for half in range(2):
                            nc.tensor.matmul(
                                rb_ps[:, half * 512:(half + 1) * 512],
                                ones1[:, :],
                                rrow[2 * h + jq][:, half * 512:(half + 1) * 512],
                                start=True, stop=True)
                        nc.vector.tensor_mul(
                            qT_h[h][:, q0:q0 + 1024],
                            oT_h[h][:, q0:q0 + 1024], rb_ps[:, :])

            # ---------------- Phase C: output projection ----------------
            with tc.tile_pool(name="psC", bufs=2, space="PSUM") as psC, \
                 tc.tile_pool(name="sbC", bufs=3) as sbC:
                for it in range(TT):
                    ps_y = [psC.tile([128, 384], F32, tag=f"y{half}",
                                     name=f"psy{half}") for half in range(2)]
                    for half in range(2):
                        ns = slice(half * 384, (half + 1) * 384)
                        for h in range(HL):
                            nc.tensor.matmul(
                                ps_y[half][:, :],
                                qT_h[h][:, it * 128:(it + 1) * 128],
                                wo_t[:, h, ns],
                                start=(h == 0), stop=(h == HL - 1))
                    y_sb = sbC.tile([128, C], F32, tag="ysb")
                    for half in range(2):
                        nc.vector.tensor_copy(
                            y_sb[:, half * 384:(half + 1) * 384], ps_y[half][:, :])
                    nc.sync.dma_start(y_d[it * 128:(it + 1) * 128, :], y_sb[:, :])

    _split_sync_waits(nc)
    return nc


def _host_inputs(x, Wq, Wdown, ln_g, ln_b, Wup, Wo):
    """Prepare the 8 per-core input maps (host-side sharding)."""
    bf = ml_dtypes.bfloat16
    # rope tables [D, T]
    inv_freq = 1.0 / (ROPE_THETA ** (np.arange(0, D, 2, dtype=np.float64) / D))
    ang = np.arange(T, dtype=np.float64)[None, :] * inv_freq[:, None]  # [D/2, T]
    ang = np.concatenate([ang, ang], axis=0)                            # [D, T]
    cosT = np.cos(ang).astype(np.float32).astype(bf)
    sinT = np.sin(ang).astype(np.float32).astype(bf)

    # rot-weight permutation: col d' of W_rot = sign * col rot_src(d')
    # rot(q)[d] = -q[d+32] (d<32) ; q[d-32] (d>=32)
    d2 = D // 2
    perm = np.concatenate([np.arange(d2, D), np.arange(0, d2)])
    sign = np.concatenate([-np.ones(d2), np.ones(d2)]).astype(np.float32)

    def rotcols(W):  # W [in, H*D] -> W_rot
        Wr = W.reshape(W.shape[0], -1, D)
        Wr = Wr[:, :, perm] * sign[None, None, :]
        return Wr.reshape(W.shape)

    Wq_k = Wq  # [C, H*D]
    Wup_k = Wup[:, 0:H * D]
    Wup_v = Wup[:, H * D:2 * H * D]
    Wq_rot = rotcols(Wq_k)
    Wup_k_rot = rotcols(Wup_k)

    mask128 = (np.arange(128)[None, :] >= np.arange(128)[:, None]).astype(np.float32)

    in_maps = []
    for core in range(N_CORES):
        b = core // 2
        hg = core % 2
        hs = slice(hg * HL * D, (hg + 1) * HL * D)
        in_maps.append({
            "xT": np.ascontiguousarray(x[b].T).astype(bf),
            "wq": Wq_k[:, hs].astype(bf),
            "wqr": Wq_rot[:, hs].astype(bf),
            "wdown": Wdown.astype(bf),
            "wupk": Wup_k[:, hs].astype(bf),
            "wupkr": Wup_k_rot[:, hs].astype(bf),
            "wupv": Wup_v[:, hs].astype(bf),
            "wo": Wo[hs, :].astype(bf),
            "cosT": cosT,
            "sinT": sinT,
            "lng": ln_g.reshape(R, 1).astype(np.float32),
            "lnb": ln_b.reshape(R, 1).astype(np.float32),
            "mask128": mask128.astype(bf),
        })
    return in_maps


def kernel(x, Wq, Wdown, ln_g, ln_b, Wup, Wo, bo):
    x = np.asarray(x, dtype=np.float32)
    Wq = np.asarray(Wq, dtype=np.float32)
    Wdown = np.asarray(Wdown, dtype=np.float32)
    ln_g = np.asarray(ln_g, dtype=np.float32)
    ln_b = np.asarray(ln_b, dtype=np.float32)
    Wup = np.asarray(Wup, dtype=np.float32)
    Wo = np.asarray(Wo, dtype=np.float32)
    bo = np.asarray(bo, dtype=np.float32)

    if "nc" not in _cached:
        _cached["nc"] = _build_nc()
    nc = _cached["nc"]

    in_maps = _host_inputs(x, Wq, Wdown, ln_g, ln_b, Wup, Wo)
    res = run_bass_kernel_spmd(nc, in_maps, core_ids=list(range(N_CORES)))

    out = np.empty((B, T, C), dtype=np.float32)
    for b in range(B):
        out[b] = res.results[2 * b]["y"] + res.results[2 * b + 1]["y"] + bo[None, :]
    return out
